# revision 6
# baseline (speedup 1.0000x reference)
"""Sliding-window GQA attention decode kernel for Trainium2 (8 NeuronCores).

Problem (hardcoded shapes): B=16, T=4, C=2048, n_head=16, n_kv_head=4,
d_head=128, S_cache=4096, sliding_window=2048, sink=4.

Sharding: hybrid tensor/data parallel over 8 cores. core = 4*b + h where
h in 0..3 is the kv-head (with its 4 grouped q-heads, column-sharded
wq/wk/wv and row-sharded w_proj) and b in 0..1 is the batch half
(8 batches each). Each core produces a partial (8,4,2048) projection
output; the host sums the 4 head-group partials per batch half.

This version is built around the measured DMA behavior: the kernel is
HBM-byte-bound, and the DMA engines only approach peak (~400GB/s/core)
on large fully-contiguous transfers kicked early with no dependency
stalls. So:
  - K and V are host-packed into their exact SBUF layouts, fully
    resident (no pair streaming): K (128, 8, 2048) d-major with
    [sink|window] cols; V (128, 8, 16, 129) tiled with the softmax
    denominator ones-column baked in. Each ships as 4 quarter
    transfers so attention on batch pair p starts as soon as quarter
    p lands.
  - All DMA kicks are issued at the top of each engine's program:
    sync ring carries x + wq (+ the tiny Vnew bounce) + wp, scalar
    ring carries tables + wkv + K, gpsimd ring carries V.
  - wk/wv ship as fp8 e4m3 scaled by 16 (they only influence the 4
    new tokens of 2052 attention positions, so the quantization is
    harmless); the 16x is undone via pre-divided RoPE tables on the
    k path and a 1/16 tensor-scalar on the v path. Everything else
    must stay fp16: fp8 on K/V/wq/wp measures ~3e-2 rel error vs the
    2e-2 budget because every contraction here is an incoherent
    random sum (error does not average out).
  - Projections are PE-instruction-lean: q proj is one matmul per
    contraction chunk (lhsT = x chunk, rhs = all 512 wq cols) into a
    (32tok, 512) psum, then 4 PE transposes + RoPE; k/v proj is fused
    into one (32, 256) psum via a concatenated [wk|wv] fp8 rhs.

Matmul operands are fp16 (fp32 matmul on trn2 is 2-pass = 4 cyc/row);
PSUM accumulation is fp32; softmax skips max-subtraction (scores ~
N(0,1); exp cannot overflow); attn^T position-major feeds attn@V as
lhsT; output partials store as fp16 and the host sums in fp32.
"""

import math

import numpy as np
import ml_dtypes

import concourse.bass as bass
import concourse.bacc as bacc
import concourse.mybir as mybir
import concourse.tile as tile
from concourse.bass_utils import run_bass_kernel_spmd

F32 = mybir.dt.float32
AF = mybir.ActivationFunctionType

MM_DT = mybir.dt.float16
MM_NP = np.float16
F8_DT = mybir.dt.float8e4
F8_NP = ml_dtypes.float8_e4m3fn

# static problem dims
B, T, C = 16, 4, 2048
NH_TOT, NKV, DH = 16, 4, 128
S_CACHE, WINDOW, SINK = 4096, 2048, 4
S = SINK + WINDOW  # 2052 attention positions per (batch, kv-head)
SC = S - T  # 2048 cached positions (sink + window-minus-new)
NT = 17  # 16 cached position tiles + 1 new-token tile
BH = B // 2  # batches per core (batch-half)
TOK = BH * T  # 32 tokens per core
NH = NH_TOT // NKV  # 4 q-heads per core (one kv-head group)
KC = C // 128  # 16 contraction tiles over C
HD = NH * DH  # 512 channels per core
WKV_SCALE = 16.0  # fp8 range scaling for wk/wv

_COMPILED = None
last_exec_time_ns = None
last_result = None


def _build_program():
    nc = bacc.Bacc("TRN2", target_bir_lowering=False, debug=False)

    xT = nc.dram_tensor("xT", [128, KC, TOK], MM_DT, kind="ExternalInput")
    wq = nc.dram_tensor("wq", [128, KC, HD], MM_DT, kind="ExternalInput")
    wkv = nc.dram_tensor("wkv", [128, KC, 2 * DH], F8_DT, kind="ExternalInput")
    wp = nc.dram_tensor("wp", [128, NH, C], MM_DT, kind="ExternalInput")
    kc = nc.dram_tensor("kc", [128, BH, SC], MM_DT, kind="ExternalInput")
    vc = nc.dram_tensor("vc", [128, BH, NT - 1, DH + 1], MM_DT, kind="ExternalInput")
    # cos/sin tables for q, then pre-divided by WKV_SCALE for k
    tabs = nc.dram_tensor("tabs", [64, 4 * TOK], F32, kind="ExternalInput")
    eye32 = nc.dram_tensor("eye32", [32, 32], MM_DT, kind="ExternalInput")
    vn_dram = nc.dram_tensor("vn_dram", [TOK, DH], MM_DT)
    outp = nc.dram_tensor("outp", [TOK, C], MM_DT, kind="ExternalOutput")

    with tile.TileContext(nc) as tc:
        with (
            tc.tile_pool(name="const", bufs=1) as cp,
            tc.tile_pool(name="tmp", bufs=2) as tp,
        ):
            xT_sb = cp.tile([128, KC, TOK], MM_DT)
            wq_sb = cp.tile([128, KC, HD], MM_DT)
            wkv_sb = cp.tile([128, KC, 2 * DH], F8_DT)
            wp_sb = cp.tile([128, NH, C], MM_DT)
            K_sb = cp.tile([128, BH, SC], MM_DT)
            V_sb = cp.tile([128, BH, NT - 1, DH + 1], MM_DT)
            tabs_sb = cp.tile([64, 4 * TOK], F32)
            eye_sb = cp.tile([32, 32], MM_DT)
            # QT columns: bb*16 + m*4 + t
            QT_sb = cp.tile([128, BH, NH, T], MM_DT)
            KnT_sb = cp.tile([128, BH, T], MM_DT)
            q_sb = cp.tile([TOK, HD], MM_DT)
            kv_sb = cp.tile([TOK, 2 * DH], MM_DT)
            Vn_sb = cp.tile([TOK, DH], MM_DT)
            # Vnew rearranged: partition = t, free = (bb, d + ones col)
            Vn2_sb = cp.tile([T, BH, DH + 1], MM_DT)
            yT_sb = cp.tile([128, NH, BH, T], MM_DT)
            vinv = cp.tile([TOK, 1], F32)

            # ---- DMA kicks: everything, up front, 3 rings ----
            # sync: x + wq (critical path to first matmul); wp appended
            # after the tiny Vnew bounce below.
            nc.sync.dma_start(xT_sb[:], xT[:])
            nc.sync.dma_start(wq_sb[:, 0:8, :], wq[:, 0:8, :])
            nc.sync.dma_start(wq_sb[:, 8:16, :], wq[:, 8:16, :])
            # scalar: tables + wkv + K quarters
            nc.scalar.dma_start(tabs_sb[:], tabs[:])
            nc.scalar.dma_start(eye_sb[:], eye32[:])
            nc.scalar.dma_start(wkv_sb[:], wkv[:])
            for qd in range(4):
                nc.scalar.dma_start(
                    K_sb[:, 2 * qd : 2 * qd + 2, :], kc[:, 2 * qd : 2 * qd + 2, :]
                )
            # gpsimd: V quarters
            for qd in range(4):
                nc.gpsimd.dma_start(
                    V_sb[:, 2 * qd : 2 * qd + 2, :, :],
                    vc[:, 2 * qd : 2 * qd + 2, :, :],
                )

            nc.vector.memset(vinv[:], 1.0 / WKV_SCALE)
            nc.vector.memset(Vn2_sb[:, :, DH : DH + 1], 1.0)

            cosq = tabs_sb[:, 0:TOK]
            sinq = tabs_sb[:, TOK : 2 * TOK]
            cosk = tabs_sb[:, 2 * TOK : 3 * TOK]
            sink = tabs_sb[:, 3 * TOK : 4 * TOK]

            def rope(dst_lo, dst_hi, src, cos_t, sin_t):
                # dst = [x1*cos - x2*sin ; x1*sin + x2*cos], halves on
                # partitions 0:64 / 64:128
                t1 = tp.tile([64, TOK], F32, tag="t1")
                t2 = tp.tile([64, TOK], F32, tag="t2")
                nc.vector.tensor_mul(t1[:], src[0:64, :], cos_t)
                nc.vector.tensor_mul(t2[:], src[64:128, :], sin_t)
                nc.vector.tensor_sub(dst_lo, t1[:], t2[:])
                t3 = tp.tile([64, TOK], F32, tag="t3")
                t4 = tp.tile([64, TOK], F32, tag="t4")
                nc.vector.tensor_mul(t3[:], src[0:64, :], sin_t)
                nc.vector.tensor_mul(t4[:], src[64:128, :], cos_t)
                nc.vector.tensor_add(dst_hi, t3[:], t4[:])

            # ---- projections: token-major psums, then PE transposes ----
            with tc.tile_pool(name="pj", bufs=3, space=bass.MemorySpace.PSUM) as pp:
                pq = pp.tile([TOK, HD], F32, tag="pj")
                for k in range(KC):
                    nc.tensor.matmul(
                        pq[:],
                        xT_sb[:, k, :],
                        wq_sb[:, k, :],
                        start=(k == 0),
                        stop=(k == KC - 1),
                    )
                nc.vector.tensor_copy(q_sb[:], pq[:])

                pkv = pp.tile([TOK, 2 * DH], F32, tag="pj")
                for k in range(KC):
                    nc.tensor.matmul(
                        pkv[:],
                        xT_sb[:, k, :],
                        wkv_sb[:, k, :],
                        start=(k == 0),
                        stop=(k == KC - 1),
                    )
                nc.vector.tensor_copy(kv_sb[:], pkv[:])
                nc.vector.tensor_scalar_mul(Vn_sb[:], kv_sb[:, DH : 2 * DH], vinv[:])

                # rearrange Vnew (4bb+t, d) -> (t, bb, d) via a DRAM bounce
                # (engine ops can't start at partition 4bb; DMA can).
                # Rides the sync ring between wq and wp.
                nc.sync.dma_start(vn_dram[:], Vn_sb[:])
                nc.sync.dma_start(
                    Vn2_sb[:, :, 0:DH], vn_dram.rearrange("(b t) d -> t b d", t=T)
                )
                nc.sync.dma_start(wp_sb[:], wp[:])

                for m in range(NH):
                    pt = pp.tile([DH, TOK], MM_DT, tag="pj")
                    nc.tensor.transpose(
                        pt[:], q_sb[:, DH * m : DH * (m + 1)], eye_sb[:]
                    )
                    rope(QT_sb[0:64, :, m, :], QT_sb[64:128, :, m, :], pt, cosq, sinq)
                ptk = pp.tile([DH, TOK], MM_DT, tag="pj")
                nc.tensor.transpose(ptk[:], kv_sb[:, 0:DH], eye_sb[:])
                rope(KnT_sb[0:64, :, :], KnT_sb[64:128, :, :], ptk, cosk, sink)

            # ---- per-batch attention ----
            with (
                tc.tile_pool(name="ax", bufs=2) as axp,
                tc.tile_pool(name="ps", bufs=2, space=bass.MemorySpace.PSUM) as psp,
                tc.tile_pool(name="py", bufs=2, space=bass.MemorySpace.PSUM) as pyp,
                tc.tile_pool(name="pyt", bufs=1, space=bass.MemorySpace.PSUM) as pytp,
                tc.tile_pool(name="po", bufs=2, space=bass.MemorySpace.PSUM) as pop,
            ):
                for b in range(BH):
                    # scoresT[s, (m,t)]: tile t at cols [16t:16t+16]
                    ps = psp.tile([128, NT, 16], F32, tag="ps")
                    for t in range(NT - 1):
                        nc.tensor.matmul(
                            ps[:, t, :],
                            K_sb[:, b, 128 * t : 128 * (t + 1)],
                            QT_sb[:, b, :, :],
                            start=True,
                            stop=True,
                        )
                    nc.tensor.matmul(
                        ps[0:T, NT - 1, :],
                        KnT_sb[:, b, :],
                        QT_sb[:, b, :, :],
                        start=True,
                        stop=True,
                    )

                    ax = axp.tile([128, NT, 16], MM_DT, tag="ax")
                    nc.scalar.activation(ax[:, 0 : NT - 1, :], ps[:, 0 : NT - 1, :], AF.Exp)
                    nc.scalar.activation(ax[0:T, NT - 1, :], ps[0:T, NT - 1, :], AF.Exp)

                    # y_aug^T: py[(m,t), 0:128]=y, py[:,128]=sum(exp)
                    py = pyp.tile([16, DH + 1], F32, tag="py")
                    for t in range(NT - 1):
                        nc.tensor.matmul(
                            py[:],
                            ax[:, t, :],
                            V_sb[:, b, t, :],
                            start=(t == 0),
                            stop=False,
                        )
                    nc.tensor.matmul(
                        py[:], ax[0:T, NT - 1, :], Vn2_sb[:, b, :], start=False, stop=True
                    )

                    rs = tp.tile([16, 1], F32, tag="rs")
                    nc.vector.reciprocal(rs[:], py[:, DH : DH + 1])
                    yn = tp.tile([16, DH], MM_DT, tag="yn")
                    nc.vector.tensor_scalar_mul(yn[:], py[:, 0:DH], rs[:])

                    pyt = pytp.tile([128, NH, T], MM_DT, tag="pyt")
                    nc.tensor.transpose(pyt[:], yn[:], eye_sb[0:16, 0:16])
                    nc.vector.tensor_copy(yT_sb[:, :, b, :], pyt[:])

                # ---- output projection (partial; host sums head groups) ----
                for n in range(4):
                    po = pop.tile([TOK, 512], F32, tag="po")
                    for kh in range(NH):
                        nc.tensor.matmul(
                            po[:],
                            yT_sb[:, kh, :, :],
                            wp_sb[:, kh, 512 * n : 512 * (n + 1)],
                            start=(kh == 0),
                            stop=(kh == NH - 1),
                        )
                    ot = tp.tile([TOK, 512], MM_DT, tag="ot")
                    if n % 2 == 0:
                        nc.vector.tensor_copy(ot[:], po[:])
                    else:
                        nc.scalar.copy(ot[:], po[:])
                    eng = nc.gpsimd if n % 2 == 0 else nc.scalar
                    eng.dma_start(outp[:, 512 * n : 512 * (n + 1)], ot[:])

    nc.compile()
    return nc


def _host_inputs(x, cache_k, cache_v, wq, wk, wv, w_proj, start_pos):
    """Build the 8 per-core input maps (host-side prep)."""
    x = np.asarray(x, dtype=np.float32)
    cache_k = np.asarray(cache_k, dtype=np.float32)
    cache_v = np.asarray(cache_v, dtype=np.float32)
    wq = np.asarray(wq, dtype=np.float32)
    wk = np.asarray(wk, dtype=np.float32)
    wv = np.asarray(wv, dtype=np.float32)
    w_proj = np.asarray(w_proj, dtype=np.float32)
    start_pos = int(np.asarray(start_pos))

    scale = np.float32(1.0 / math.sqrt(DH))

    # RoPE tables at absolute positions [start_pos, start_pos+T)
    half = DH // 2
    inv_freq = (
        1.0 / (10000.0 ** (np.arange(half, dtype=np.float32) / np.float32(half)))
    ).astype(np.float32)
    pos = np.arange(start_pos, start_pos + T, dtype=np.float32)
    ang = pos[:, None] * inv_freq[None, :]  # (T, 64)
    cos4 = np.cos(ang).astype(np.float32).T  # (64, T)
    sin4 = np.sin(ang).astype(np.float32).T
    cos_t = np.ascontiguousarray(np.tile(cos4, (1, BH)))  # (64, TOK), col=bb*T+t
    sin_t = np.ascontiguousarray(np.tile(sin4, (1, BH)))
    tabs = np.concatenate(
        [cos_t, sin_t, cos_t / WKV_SCALE, sin_t / WKV_SCALE], axis=1
    ).astype(np.float32)
    eye32 = np.eye(32, dtype=MM_NP)

    # sliding-window + sink slice of the caches: positions [0:4] + [2052:4096]
    lo = S_CACHE - (WINDOW - T)
    kt = np.concatenate([cache_k[:, :, :SINK, :], cache_k[:, :, lo:, :]], axis=2)
    vt = np.concatenate([cache_v[:, :, :SINK, :], cache_v[:, :, lo:, :]], axis=2)
    # K d-major per (batch, head): (B, NKV, DH, SC)
    ktT = kt.transpose(0, 1, 3, 2).astype(MM_NP)
    # V tiled (B, NKV, 128, 16, 129) with ones column baked in
    vtile = np.empty((B, NKV, 128, NT - 1, DH + 1), dtype=MM_NP)
    vtile[..., :DH] = vt.reshape(B, NKV, NT - 1, 128, DH).transpose(0, 1, 3, 2, 4)
    vtile[..., DH] = np.float16(1.0)

    wq_s = (wq * scale).astype(MM_NP)
    wp_h = w_proj.astype(MM_NP)

    def tile_w(w, dt):
        # (rows, cols) -> (128, rows/128, cols), contiguous
        r, c = w.shape
        return np.ascontiguousarray(w.reshape(r // 128, 128, c).transpose(1, 0, 2)).astype(dt)

    in_maps = []
    for core in range(8):
        h, bb = core % NKV, core // NKV
        sl = slice(BH * bb, BH * (bb + 1))
        wkv_h = np.concatenate(
            [wk[:, DH * h : DH * (h + 1)], wv[:, DH * h : DH * (h + 1)]], axis=1
        ) * WKV_SCALE
        in_maps.append(
            {
                "xT": np.ascontiguousarray(
                    x[sl].reshape(TOK, KC, 128).transpose(2, 1, 0)
                ).astype(MM_NP),
                "wq": tile_w(wq_s[:, HD * h : HD * (h + 1)], MM_NP),
                "wkv": tile_w(wkv_h, F8_NP),
                "wp": tile_w(wp_h[HD * h : HD * (h + 1), :], MM_NP),
                "kc": np.ascontiguousarray(ktT[sl, h].transpose(1, 0, 2)),
                "vc": np.ascontiguousarray(vtile[sl, h].transpose(1, 0, 2, 3)),
                "tabs": tabs,
                "eye32": eye32,
            }
        )
    return in_maps


def kernel(x, cache_k, cache_v, wq, wk, wv, w_proj, start_pos):
    global _COMPILED, last_exec_time_ns, last_result
    if _COMPILED is None:
        _COMPILED = _build_program()
    nc = _COMPILED

    in_maps = _host_inputs(x, cache_k, cache_v, wq, wk, wv, w_proj, start_pos)
    res = run_bass_kernel_spmd(nc, in_maps, core_ids=list(range(8)))
    last_exec_time_ns = res.exec_time_ns
    last_result = res

    out = np.zeros((B, T, C), dtype=np.float32)
    for core in range(8):
        h, bb = core % NKV, core // NKV
        out[BH * bb : BH * (bb + 1)] += (
            res.results[core]["outp"].astype(np.float32).reshape(BH, T, C)
        )
    return out


# revision 9
# speedup vs baseline: 1.1747x; 1.1747x over previous
"""Sliding-window GQA attention decode kernel for Trainium2 (8 NeuronCores).

Problem (hardcoded shapes): B=16, T=4, C=2048, n_head=16, n_kv_head=4,
d_head=128, S_cache=4096, sliding_window=2048, sink=4.

Sharding: hybrid tensor/data parallel over 8 cores. core = 4*b + h where
h in 0..3 is the kv-head (with its 4 grouped q-heads, column-sharded
wq/wk/wv and row-sharded w_proj) and b in 0..1 is the batch half
(8 batches each). Each core produces a partial (8,4,2048) projection
output; the host sums the 4 head-group partials per batch half.

This version is built around the measured DMA behavior: the kernel is
HBM-byte-bound, and the DMA engines only approach peak (~400GB/s/core)
on large fully-contiguous transfers kicked early with no dependency
stalls. So:
  - K and V are host-packed into their exact SBUF layouts, fully
    resident (no pair streaming): K (128, 8, 2048) d-major with
    [sink|window] cols; V (128, 8, 16, 129) tiled with the softmax
    denominator ones-column baked in. Each ships as 4 quarter
    transfers so attention on batch pair p starts as soon as quarter
    p lands.
  - All DMA kicks are issued at the top of each engine's program:
    sync ring carries x + wq (+ the tiny Vnew bounce) + wp, scalar
    ring carries tables + wkv + K, gpsimd ring carries V.
  - wk/wv ship as fp8 e4m3 scaled by 16 (they only influence the 4
    new tokens of 2052 attention positions, so the quantization is
    harmless); the 16x is undone via pre-divided RoPE tables on the
    k path and a 1/16 tensor-scalar on the v path. Everything else
    must stay fp16: fp8 on K/V/wq/wp measures ~3e-2 rel error vs the
    2e-2 budget because every contraction here is an incoherent
    random sum (error does not average out).
  - Projections are PE-instruction-lean: q proj is one matmul per
    contraction chunk (lhsT = x chunk, rhs = all 512 wq cols) into a
    (32tok, 512) psum, then 4 PE transposes + RoPE; k/v proj is fused
    into one (32, 256) psum via a concatenated [wk|wv] fp8 rhs.

Matmul operands are fp16 (fp32 matmul on trn2 is 2-pass = 4 cyc/row);
PSUM accumulation is fp32; softmax skips max-subtraction (scores ~
N(0,1); exp cannot overflow); attn^T position-major feeds attn@V as
lhsT; output partials store as fp16 and the host sums in fp32.
"""

import math

import numpy as np
import ml_dtypes

import concourse.bass as bass
import concourse.bacc as bacc
import concourse.mybir as mybir
import concourse.tile as tile
from concourse.bass_utils import run_bass_kernel_spmd

F32 = mybir.dt.float32
AF = mybir.ActivationFunctionType

MM_DT = mybir.dt.float16
MM_NP = np.float16
F8_DT = mybir.dt.float8e4
F8_NP = ml_dtypes.float8_e4m3fn

# static problem dims
B, T, C = 16, 4, 2048
NH_TOT, NKV, DH = 16, 4, 128
S_CACHE, WINDOW, SINK = 4096, 2048, 4
S = SINK + WINDOW  # 2052 attention positions per (batch, kv-head)
SC = S - T  # 2048 cached positions (sink + window-minus-new)
NT = 17  # 16 cached position tiles + 1 new-token tile
BH = B // 2  # batches per core (batch-half)
TOK = BH * T  # 32 tokens per core
NH = NH_TOT // NKV  # 4 q-heads per core (one kv-head group)
KC = C // 128  # 16 contraction tiles over C
HD = NH * DH  # 512 channels per core
WKV_SCALE = 16.0  # fp8 range scaling for wk/wv

_COMPILED = None
last_exec_time_ns = None
last_result = None


def _build_program():
    nc = bacc.Bacc("TRN2", target_bir_lowering=False, debug=False)

    xT = nc.dram_tensor("xT", [128, KC, TOK], MM_DT, kind="ExternalInput")
    wq = nc.dram_tensor("wq", [128, KC, HD], MM_DT, kind="ExternalInput")
    wkv = nc.dram_tensor("wkv", [128, KC, 2 * DH], F8_DT, kind="ExternalInput")
    wp = nc.dram_tensor("wp", [128, NH, C], MM_DT, kind="ExternalInput")
    kc = nc.dram_tensor("kc", [128, BH, SC], MM_DT, kind="ExternalInput")
    vc = nc.dram_tensor("vc", [128, BH, NT - 1, DH + 1], MM_DT, kind="ExternalInput")
    # cos/sin tables for q, then pre-divided by WKV_SCALE for k
    tabs = nc.dram_tensor("tabs", [64, 4 * TOK], F32, kind="ExternalInput")
    eye32 = nc.dram_tensor("eye32", [32, 32], MM_DT, kind="ExternalInput")
    vn_dram = nc.dram_tensor("vn_dram", [TOK, DH], MM_DT)
    outp = nc.dram_tensor("outp", [TOK, C], MM_DT, kind="ExternalOutput")

    with tile.TileContext(nc) as tc:
        with (
            tc.tile_pool(name="const", bufs=1) as cp,
            tc.tile_pool(name="tmp", bufs=2) as tp,
        ):
            xT_sb = cp.tile([128, KC, TOK], MM_DT)
            wq_sb = cp.tile([128, KC, HD], MM_DT)
            wkv_sb = cp.tile([128, KC, 2 * DH], F8_DT)
            wp_sb = cp.tile([128, NH, C], MM_DT)
            K_sb = cp.tile([128, BH, SC], MM_DT)
            V_sb = cp.tile([128, BH, NT - 1, DH + 1], MM_DT)
            tabs_sb = cp.tile([64, 4 * TOK], F32)
            eye_sb = cp.tile([32, 32], MM_DT)
            # QT columns: bb*16 + m*4 + t
            QT_sb = cp.tile([128, BH, NH, T], MM_DT)
            KnT_sb = cp.tile([128, BH, T], MM_DT)
            q_sb = cp.tile([TOK, HD], MM_DT)
            kv_sb = cp.tile([TOK, 2 * DH], MM_DT)
            Vn_sb = cp.tile([TOK, DH], MM_DT)
            # Vnew rearranged: partition = t, free = (bb, d + ones col)
            Vn2_sb = cp.tile([T, BH, DH + 1], MM_DT)
            yT_sb = cp.tile([128, NH, BH, T], MM_DT)
            vinv = cp.tile([TOK, 1], F32)

            # ---- DMA kicks: ONE ring (sync), in need-order FIFO.
            # Concurrent queues split the ~415GB/s per-core DMA rate
            # (3 queues measure ~100GB/s each), so a single need-ordered
            # ring is strictly better. Only the tiny Vnew bounce (scalar
            # ring) and the output stores (gpsimd ring) go elsewhere.
            nc.sync.dma_start(xT_sb[:], xT[:])
            nc.sync.dma_start(wq_sb[:, 0:8, :], wq[:, 0:8, :])
            nc.sync.dma_start(wq_sb[:, 8:16, :], wq[:, 8:16, :])
            nc.sync.dma_start(wkv_sb[:], wkv[:])
            nc.sync.dma_start(tabs_sb[:], tabs[:])
            nc.sync.dma_start(eye_sb[:], eye32[:])
            for qd in range(4):
                nc.sync.dma_start(
                    K_sb[:, 2 * qd : 2 * qd + 2, :], kc[:, 2 * qd : 2 * qd + 2, :]
                )
                nc.sync.dma_start(
                    V_sb[:, 2 * qd : 2 * qd + 2, :, :],
                    vc[:, 2 * qd : 2 * qd + 2, :, :],
                )
            nc.sync.dma_start(wp_sb[:, 0:2, :], wp[:, 0:2, :])
            nc.sync.dma_start(wp_sb[:, 2:4, :], wp[:, 2:4, :])

            nc.vector.memset(vinv[:], 1.0 / WKV_SCALE)
            nc.vector.memset(Vn2_sb[:, :, DH : DH + 1], 1.0)

            cosq = tabs_sb[:, 0:TOK]
            sinq = tabs_sb[:, TOK : 2 * TOK]
            cosk = tabs_sb[:, 2 * TOK : 3 * TOK]
            sink = tabs_sb[:, 3 * TOK : 4 * TOK]

            def rope(dst_lo, dst_hi, src, cos_t, sin_t):
                # dst = [x1*cos - x2*sin ; x1*sin + x2*cos], halves on
                # partitions 0:64 / 64:128
                t1 = tp.tile([64, TOK], F32, tag="t1")
                t2 = tp.tile([64, TOK], F32, tag="t2")
                nc.vector.tensor_mul(t1[:], src[0:64, :], cos_t)
                nc.vector.tensor_mul(t2[:], src[64:128, :], sin_t)
                nc.vector.tensor_sub(dst_lo, t1[:], t2[:])
                t3 = tp.tile([64, TOK], F32, tag="t3")
                t4 = tp.tile([64, TOK], F32, tag="t4")
                nc.vector.tensor_mul(t3[:], src[0:64, :], sin_t)
                nc.vector.tensor_mul(t4[:], src[64:128, :], cos_t)
                nc.vector.tensor_add(dst_hi, t3[:], t4[:])

            # ---- projections: token-major psums, then PE transposes ----
            with tc.tile_pool(name="pj", bufs=3, space=bass.MemorySpace.PSUM) as pp:
                pq = pp.tile([TOK, HD], F32, tag="pj")
                for k in range(KC):
                    nc.tensor.matmul(
                        pq[:],
                        xT_sb[:, k, :],
                        wq_sb[:, k, :],
                        start=(k == 0),
                        stop=(k == KC - 1),
                    )
                nc.vector.tensor_copy(q_sb[:], pq[:])

                pkv = pp.tile([TOK, 2 * DH], F32, tag="pj")
                for k in range(KC):
                    nc.tensor.matmul(
                        pkv[:],
                        xT_sb[:, k, :],
                        wkv_sb[:, k, :],
                        start=(k == 0),
                        stop=(k == KC - 1),
                    )
                nc.vector.tensor_copy(kv_sb[:], pkv[:])
                nc.vector.tensor_scalar_mul(Vn_sb[:], kv_sb[:, DH : 2 * DH], vinv[:])

                # rearrange Vnew (4bb+t, d) -> (t, bb, d) via a DRAM bounce
                # (engine ops can't start at partition 4bb; DMA can).
                # Scalar ring: tiny, and must not queue behind K/V/wp.
                nc.scalar.dma_start(vn_dram[:], Vn_sb[:])
                nc.scalar.dma_start(
                    Vn2_sb[:, :, 0:DH], vn_dram.rearrange("(b t) d -> t b d", t=T)
                )

                for m in range(NH):
                    pt = pp.tile([DH, TOK], MM_DT, tag="pj")
                    nc.tensor.transpose(
                        pt[:], q_sb[:, DH * m : DH * (m + 1)], eye_sb[:]
                    )
                    rope(QT_sb[0:64, :, m, :], QT_sb[64:128, :, m, :], pt, cosq, sinq)
                ptk = pp.tile([DH, TOK], MM_DT, tag="pj")
                nc.tensor.transpose(ptk[:], kv_sb[:, 0:DH], eye_sb[:])
                rope(KnT_sb[0:64, :, :], KnT_sb[64:128, :, :], ptk, cosk, sink)

            # ---- per-batch attention, software-pipelined ----
            # PE issue order: S0, S1, A0, S2, T0, A1, S3, T1, ... so the
            # PE never sits in an exp/norm dependency wait: scores for
            # batch b+2 run while scalar does exp_{b+1}, and the yT
            # transpose of b runs after vector finished normalizing b.
            with (
                tc.tile_pool(name="ax", bufs=3) as axp,
                tc.tile_pool(name="ps", bufs=3, space=bass.MemorySpace.PSUM) as psp,
                tc.tile_pool(name="py", bufs=2, space=bass.MemorySpace.PSUM) as pyp,
                tc.tile_pool(name="pyt", bufs=1, space=bass.MemorySpace.PSUM) as pytp,
                tc.tile_pool(name="po", bufs=2, space=bass.MemorySpace.PSUM) as pop,
            ):
                pss, axs = {}, {}

                def scores(b):
                    # scoresT[s, (m,t)]: tile t at cols [16t:16t+16]
                    ps = psp.tile([128, NT, 16], F32, tag="ps", name=f"ps{b}")
                    pss[b] = ps
                    for t in range(NT - 1):
                        nc.tensor.matmul(
                            ps[:, t, :],
                            K_sb[:, b, 128 * t : 128 * (t + 1)],
                            QT_sb[:, b, :, :],
                            start=True,
                            stop=True,
                        )
                    nc.tensor.matmul(
                        ps[0:T, NT - 1, :],
                        KnT_sb[:, b, :],
                        QT_sb[:, b, :, :],
                        start=True,
                        stop=True,
                    )
                    ax = axp.tile([128, NT, 16], MM_DT, tag="ax", name=f"ax{b}")
                    axs[b] = ax
                    nc.scalar.activation(ax[:, 0 : NT - 1, :], ps[:, 0 : NT - 1, :], AF.Exp)
                    nc.scalar.activation(ax[0:T, NT - 1, :], ps[0:T, NT - 1, :], AF.Exp)

                scores(0)
                scores(1)
                for b in range(BH):
                    ax = axs.pop(b)
                    pss.pop(b)
                    # y_aug^T: py[(m,t), 0:128]=y, py[:,128]=sum(exp)
                    py = pyp.tile([16, DH + 1], F32, tag="py")
                    for t in range(NT - 1):
                        nc.tensor.matmul(
                            py[:],
                            ax[:, t, :],
                            V_sb[:, b, t, :],
                            start=(t == 0),
                            stop=False,
                        )
                    nc.tensor.matmul(
                        py[:], ax[0:T, NT - 1, :], Vn2_sb[:, b, :], start=False, stop=True
                    )

                    rs = tp.tile([16, 1], F32, tag="rs")
                    nc.vector.reciprocal(rs[:], py[:, DH : DH + 1])
                    yn = tp.tile([16, DH], MM_DT, tag="yn")
                    nc.vector.tensor_scalar_mul(yn[:], py[:, 0:DH], rs[:])

                    if b + 2 < BH:
                        scores(b + 2)

                    pyt = pytp.tile([128, NH, T], MM_DT, tag="pyt")
                    nc.tensor.transpose(pyt[:], yn[:], eye_sb[0:16, 0:16])
                    nc.vector.tensor_copy(yT_sb[:, :, b, :], pyt[:])

                # ---- output projection (partial; host sums head groups) ----
                for n in range(4):
                    po = pop.tile([TOK, 512], F32, tag="po")
                    for kh in range(NH):
                        nc.tensor.matmul(
                            po[:],
                            yT_sb[:, kh, :, :],
                            wp_sb[:, kh, 512 * n : 512 * (n + 1)],
                            start=(kh == 0),
                            stop=(kh == NH - 1),
                        )
                    ot = tp.tile([TOK, 512], MM_DT, tag="ot")
                    if n % 2 == 0:
                        nc.vector.tensor_copy(ot[:], po[:])
                    else:
                        nc.scalar.copy(ot[:], po[:])
                    eng = nc.gpsimd if n % 2 == 0 else nc.scalar
                    eng.dma_start(outp[:, 512 * n : 512 * (n + 1)], ot[:])

    nc.compile()
    return nc


def _host_inputs(x, cache_k, cache_v, wq, wk, wv, w_proj, start_pos):
    """Build the 8 per-core input maps (host-side prep)."""
    x = np.asarray(x, dtype=np.float32)
    cache_k = np.asarray(cache_k, dtype=np.float32)
    cache_v = np.asarray(cache_v, dtype=np.float32)
    wq = np.asarray(wq, dtype=np.float32)
    wk = np.asarray(wk, dtype=np.float32)
    wv = np.asarray(wv, dtype=np.float32)
    w_proj = np.asarray(w_proj, dtype=np.float32)
    start_pos = int(np.asarray(start_pos))

    scale = np.float32(1.0 / math.sqrt(DH))

    # RoPE tables at absolute positions [start_pos, start_pos+T)
    half = DH // 2
    inv_freq = (
        1.0 / (10000.0 ** (np.arange(half, dtype=np.float32) / np.float32(half)))
    ).astype(np.float32)
    pos = np.arange(start_pos, start_pos + T, dtype=np.float32)
    ang = pos[:, None] * inv_freq[None, :]  # (T, 64)
    cos4 = np.cos(ang).astype(np.float32).T  # (64, T)
    sin4 = np.sin(ang).astype(np.float32).T
    cos_t = np.ascontiguousarray(np.tile(cos4, (1, BH)))  # (64, TOK), col=bb*T+t
    sin_t = np.ascontiguousarray(np.tile(sin4, (1, BH)))
    tabs = np.concatenate(
        [cos_t, sin_t, cos_t / WKV_SCALE, sin_t / WKV_SCALE], axis=1
    ).astype(np.float32)
    eye32 = np.eye(32, dtype=MM_NP)

    # sliding-window + sink slice of the caches: positions [0:4] + [2052:4096]
    lo = S_CACHE - (WINDOW - T)
    kt = np.concatenate([cache_k[:, :, :SINK, :], cache_k[:, :, lo:, :]], axis=2)
    vt = np.concatenate([cache_v[:, :, :SINK, :], cache_v[:, :, lo:, :]], axis=2)
    # K d-major per (batch, head): (B, NKV, DH, SC)
    ktT = kt.transpose(0, 1, 3, 2).astype(MM_NP)
    # V tiled (B, NKV, 128, 16, 129) with ones column baked in
    vtile = np.empty((B, NKV, 128, NT - 1, DH + 1), dtype=MM_NP)
    vtile[..., :DH] = vt.reshape(B, NKV, NT - 1, 128, DH).transpose(0, 1, 3, 2, 4)
    vtile[..., DH] = np.float16(1.0)

    wq_s = (wq * scale).astype(MM_NP)
    wp_h = w_proj.astype(MM_NP)

    def tile_w(w, dt):
        # (rows, cols) -> (128, rows/128, cols), contiguous
        r, c = w.shape
        return np.ascontiguousarray(w.reshape(r // 128, 128, c).transpose(1, 0, 2)).astype(dt)

    in_maps = []
    for core in range(8):
        h, bb = core % NKV, core // NKV
        sl = slice(BH * bb, BH * (bb + 1))
        wkv_h = np.concatenate(
            [wk[:, DH * h : DH * (h + 1)], wv[:, DH * h : DH * (h + 1)]], axis=1
        ) * WKV_SCALE
        in_maps.append(
            {
                "xT": np.ascontiguousarray(
                    x[sl].reshape(TOK, KC, 128).transpose(2, 1, 0)
                ).astype(MM_NP),
                "wq": tile_w(wq_s[:, HD * h : HD * (h + 1)], MM_NP),
                "wkv": tile_w(wkv_h, F8_NP),
                "wp": tile_w(wp_h[HD * h : HD * (h + 1), :], MM_NP),
                "kc": np.ascontiguousarray(ktT[sl, h].transpose(1, 0, 2)),
                "vc": np.ascontiguousarray(vtile[sl, h].transpose(1, 0, 2, 3)),
                "tabs": tabs,
                "eye32": eye32,
            }
        )
    return in_maps


def kernel(x, cache_k, cache_v, wq, wk, wv, w_proj, start_pos):
    global _COMPILED, last_exec_time_ns, last_result
    if _COMPILED is None:
        _COMPILED = _build_program()
    nc = _COMPILED

    in_maps = _host_inputs(x, cache_k, cache_v, wq, wk, wv, w_proj, start_pos)
    res = run_bass_kernel_spmd(nc, in_maps, core_ids=list(range(8)))
    last_exec_time_ns = res.exec_time_ns
    last_result = res

    out = np.zeros((B, T, C), dtype=np.float32)
    for core in range(8):
        h, bb = core % NKV, core // NKV
        out[BH * bb : BH * (bb + 1)] += (
            res.results[core]["outp"].astype(np.float32).reshape(BH, T, C)
        )
    return out
